# revision 1
# baseline (speedup 1.0000x reference)
"""AttentionPairBias distributed Trainium2 kernel.

Sharding: the 1024 query rows (i) are split across 8 NeuronCores, 128 rows
each.  z_ij is sharded over i and fed to each core pre-transposed to
[c_z, i*N+j] layout so the pair-bias projection can stream on the tensor
engine.  All per-core programs are identical (SPMD): the per-core i-offset is
absorbed host-side by rotating the token axis of a_i / s_i / z's j axis,
which softmax/attention results are invariant to.

Device-side math (per core, fp32 throughout):
  AdaLN   a = sigmoid(ln(s) @ ws' + bs') * ln(a) + ln(s) @ wns'
          (per-channel adaln_lns_w folded into ws'/wns' host-side)
  QKV     computed in transposed layout [hd, tok] via lhsT=weight,
          rhs=a^T; heads padded 24 -> 32 so per-head slices are 32-aligned.
  Pair bias  braw[h, ij] = sum_c z[c, ij] * wb''[c, h] with
          wb'' = (lnb_w*wb) - mean_c(lnb_w*wb): the LN mean-centering folds
          into the projection weights.  The lnb_b @ wb term is constant over
          j and drops out of softmax.  Row 16 = column sum of z (-> mean),
          row 17 = column sum of z^2 (-> var).  bias = rs * braw with
          rs = 1/sqrt(var + eps).
  The [18, ij] stats block round-trips through DRAM; reading it back as
  [i, j] planes per head performs the partition transpose for free.
  Attention: logits psum [i, 512 j] per head; Exp with accum_out yields the
  softmax denominator; A = E * (1/den), PE-transpose feeds A^T into the AV
  matmul producing o^T; gate, output projection and the final s-gate finish
  in natural layout.
"""

import os
import sys

import numpy as np

for _p in ("/opt/trn_rl_repo",):
    if _p not in sys.path and os.path.isdir(_p):
        sys.path.insert(0, _p)

import concourse.bass as bass
import concourse.tile as tile
from concourse import mybir
from concourse.bass_utils import run_bass_kernel_spmd
from concourse.masks import make_identity
from concourse.tile import add_dep_helper

# ---------------------------------------------------------------------------
# The walrus build in this container accepts at most ONE sync-wait command per
# instruction, while current Tile emits multi-wait sync_info.  Patch the BIR
# just before compilation: extra waits move onto preceding same-engine NoOps
# (sequencer executes them in order, so semantics are identical).
import json as _json

import concourse.bass_utils as _bass_utils
import concourse.bass2jax as _bass2jax

_ORIG_COMPILE_BIR = _bass_utils.compile_bir_kernel


def _split_sync_waits(bir_json, max_waits=1):
    d = _json.loads(bir_json)
    ctr = 0
    for fn in d["functions"]:
        for bb in fn["blocks"]:
            new = []
            for inst in bb["instructions"]:
                si = inst.get("sync_info")
                if si and si.get("on_wait") and len(si["on_wait"]) > max_waits:
                    waits = si["on_wait"]
                    extra, keep = waits[:-max_waits], waits[-max_waits:]
                    for w in extra:
                        ctr += 1
                        new.append({
                            "debug": inst.get("debug", 0),
                            "engine": inst["engine"],
                            "ins": [], "outs": [],
                            "name": f"WSPL-{ctr}",
                            "opcode": "NoOp",
                            "sync_info": {"on_update": [], "on_wait": [w]},
                        })
                    si["on_wait"] = keep
                new.append(inst)
            bb["instructions"] = new
    return _json.dumps(d).encode()


def _patched_compile_bir_kernel(bir_json, tmpdir, neff_name="file.neff"):
    return _ORIG_COMPILE_BIR(_split_sync_waits(bir_json), tmpdir, neff_name)


_bass_utils.compile_bir_kernel = _patched_compile_bir_kernel
_bass2jax.compile_bir_kernel = _patched_compile_bir_kernel
# ---------------------------------------------------------------------------

# Problem shape (hardcoded per contract).  N/NCORES overridable for sim tests.
B, C_S, C_Z, H, D = 1, 384, 128, 16, 24
N = 1024
NCORES = 8
DP = 32                   # padded head dim
HDP = H * DP              # 512
KC = C_S // 128           # 3 contraction chunks over c_s
MC4 = HDP // 128          # 4 chunks over padded heads
ZCH = 2048                # ij columns per z DMA chunk
NMM = 512                 # fp32 moving-operand width
EPS = 1e-5
SCALE = 1.0 / float(np.sqrt(np.float32(D)))


def _derive():
    global IB, IJ, NZC, NJC, LW, NHALF
    IB = N // NCORES      # 128 query rows per core (must stay 128)
    IJ = IB * N           # i-major ij index space per core
    NZC = IJ // ZCH       # z chunks
    NJC = N // 128        # j chunks
    LW = min(NMM, N)      # logits matmul width
    NHALF = N // LW
    assert IB == 128 and IJ % ZCH == 0 and NZC % 4 == 0


_derive()


def _set_test_size(n, ncores):
    """Shrink the problem for CoreSim tests (keeps IB=128)."""
    global N, NCORES
    N, NCORES = n, ncores
    _derive()
    _CACHED.clear()

f32 = mybir.dt.float32
f32r = mybir.dt.float32r
AF = mybir.ActivationFunctionType
ALU = mybir.AluOpType
USE_F32R = os.environ.get("KERNEL_FP32R", "1") == "1"
USE_ZBF16 = True
bf16 = mybir.dt.bfloat16


def _R(ap):
    """Reinterpret an fp32 AP as float32r: single-pass (1 cycle/column)
    matmul mode vs 4 cycles/column for plain fp32, at slightly reduced
    multiply precision.  Only applied to wide (N>=256) matmuls."""
    return ap.bitcast(f32r) if USE_F32R else ap

_CACHED = {}


def _build_program():
    nc = bass.Bass()
    p = {}
    fr = f32r if USE_F32R else f32
    zdt = bf16 if USE_ZBF16 else fr
    decl = [
        ("z_t", [C_Z, IJ], zdt), ("a_in", [N, C_S], f32), ("s_in", [N, C_S], f32),
        ("w_ws", [C_S, C_S], fr), ("w_wns", [C_S, C_S], fr), ("b_s", [C_S], f32),
        ("w_q", [C_S, HDP], fr), ("b_q", [HDP], f32), ("w_k", [C_S, HDP], fr),
        ("w_v", [C_S, HDP], fr), ("w_g", [C_S, HDP], fr),
        ("wb_aug", [C_Z, 32], zdt), ("sq_aug", [C_Z, 32], zdt),
        ("w_o", [HDP, C_S], fr), ("w_sg", [C_S, C_S], fr), ("b_sg", [C_S], f32),
    ]
    for name, shape, dt_ in decl:
        p[name] = nc.declare_dram_parameter(name, shape, dt_, isOutput=False)
    p["out"] = nc.declare_dram_parameter("out", [IB, C_S], f32, isOutput=True)

    with tile.TileContext(nc) as tc:
        _emit(tc, p)
    return nc


def _emit(tc, p):
    from contextlib import ExitStack

    nc = tc.nc
    fr = f32r if USE_F32R else f32
    zdt = bf16 if USE_ZBF16 else fr
    ctx = ExitStack()
    with ctx:
        singles = ctx.enter_context(tc.tile_pool(name="singles", bufs=1))
        persist = ctx.enter_context(tc.tile_pool(name="persist", bufs=1))
        dram = ctx.enter_context(tc.tile_pool(name="dram", bufs=1, space="DRAM"))
        ps_stat = ctx.enter_context(tc.tile_pool(name="ps_stat", bufs=2, space="PSUM"))
        ps_big = ctx.enter_context(tc.tile_pool(name="ps_big", bufs=2, space="PSUM"))
        ps_sm = ctx.enter_context(tc.tile_pool(name="ps_sm", bufs=2, space="PSUM"))
        ps_o = ctx.enter_context(tc.tile_pool(name="ps_o", bufs=2, space="PSUM"))

        def pbig():
            return ps_big.tile([128, NMM], f32, name="psb", tag="psb")

        def psm():
            return ps_sm.tile([128, 128], f32, name="pss", tag="pss")

        _alt = [0]
        ET = mybir.EngineType
        cur_nops = {}
        dma_log = []

        def dma(out, in_, **kw):
            i = nc.sync.dma_start(out=out, in_=in_, **kw)
            dma_log.append(i.ins)
            return i

        def sync_point():
            # Full barrier, then one nop per engine that absorbs the
            # accumulated per-DMA-lane waits so later instructions (esp.
            # matmuls, whose encoding has few sync-wait slots) stay cheap.
            tc.strict_bb_all_engine_barrier()
            bar = tc.barrier_instruction_and_bb[0]
            cur_nops.clear()
            for eng in (ET.PE, ET.DVE, ET.Activation, ET.Pool):
                nop = nc.engines[eng].nop(nofuse=True, hint="absorb")
                add_dep_helper(nop.ins, bar, reason="phase sync absorber")
                for d in dma_log:
                    add_dep_helper(nop.ins, d, reason="absorb dma lanes")
                cur_nops[eng] = nop.ins
            del dma_log[:]

        def pin(bass_inst, eng):
            # Order bass_inst after the current absorber nop on its engine
            # (same-engine dep: pure ordering, no semaphore cost).
            if eng in cur_nops:
                add_dep_helper(bass_inst.ins, cur_nops[eng], reason="pin")

        def copy_alt(out, in_, pin_it=False):
            # alternate PSUM->SBUF copies between DVE and ACT
            _alt[0] ^= 1
            if _alt[0]:
                i = nc.vector.tensor_copy(out=out, in_=in_)
                if pin_it:
                    pin(i, ET.DVE)
            else:
                i = nc.scalar.activation(out=out, in_=in_, func=AF.Copy)
                if pin_it:
                    pin(i, ET.Activation)
            return i

        ident = singles.tile([128, 128], f32, tag="ident")
        make_identity(nc, ident)
        eps_t = singles.tile([128, 1], f32, tag="eps")
        nc.vector.memset(eps_t, EPS)

        # ---- weights to SBUF ----
        def wload(name, ap, shape):
            w = singles.tile(shape, ap.dtype, name=name, tag=name)
            dma(out=w, in_=ap)
            return w

        r3 = "(kc pp) o -> pp kc o"
        ws_sb = wload("ws_sb", p["w_ws"][:, :].rearrange(r3, pp=128), [128, KC, C_S])
        wns_sb = wload("wns_sb", p["w_wns"][:, :].rearrange(r3, pp=128), [128, KC, C_S])
        wq_sb = wload("wq_sb", p["w_q"][:, :].rearrange(r3, pp=128), [128, KC, HDP])
        wk_sb = wload("wk_sb", p["w_k"][:, :].rearrange(r3, pp=128), [128, KC, HDP])
        wv_sb = wload("wv_sb", p["w_v"][:, :].rearrange(r3, pp=128), [128, KC, HDP])
        wg_sb = wload("wg_sb", p["w_g"][:, :].rearrange(r3, pp=128), [128, KC, HDP])
        wo_sb = wload("wo_sb", p["w_o"][:, :].rearrange(r3, pp=128), [128, MC4, C_S])
        wsg_sb = wload("wsg_sb", p["w_sg"][:, :].rearrange(r3, pp=128), [128, KC, C_S])
        wba_sb = wload("wba_sb", p["wb_aug"][:, :], [C_Z, 32])
        sqa_sb = wload("sqa_sb", p["sq_aug"][:, :], [C_Z, 32])
        bs_sb = wload("bs_sb", p["b_s"][:].rearrange("(mc pp) -> pp mc", pp=128), [128, KC])
        bq_sb = wload("bq_sb", p["b_q"][:].rearrange("(mc pp) -> pp mc", pp=128), [128, MC4])
        # b_sg broadcast across partitions for the natural-layout final gate
        bsg_ap = p["b_sg"][:]
        bsg_bc = singles.tile([128, C_S], f32, tag="bsg_bc")
        dma(
            out=bsg_bc,
            in_=bass.AP(tensor=bsg_ap.tensor, offset=bsg_ap.offset,
                        ap=[[0, 128]] + [list(d) for d in bsg_ap.ap]),
        )

        braw = dram.tile([128, IJ // 4], bf16)

        # ============ Phase B: AdaLN + projections ============
        kT = [persist.tile([128, N], fr, name=f"kT{m}", tag=f"kT{m}") for m in range(MC4)]
        V = [persist.tile([128, HDP], f32, name=f"V{j}", tag=f"V{j}") for j in range(NJC)]
        qT = [persist.tile([128, IB], fr, name=f"qT{m}", tag=f"qT{m}") for m in range(MC4)]
        gT = [persist.tile([128, IB], f32, name=f"gT{m}", tag=f"gT{m}") for m in range(MC4)]
        siT = [persist.tile([128, IB], fr, name=f"siT{k}", tag=f"siT{k}") for k in range(KC)]

        with tc.tile_pool(name="adaln", bufs=3) as ad, \
             tc.tile_pool(name="adbuf", bufs=1) as adb:
            sT = [adb.tile([128, N], fr, name=f"sT{k}", tag=f"sT{k}") for k in range(KC)]
            lnaT = [adb.tile([128, N], f32, name=f"lnaT{k}", tag=f"lnaT{k}") for k in range(KC)]
            aT = [adb.tile([128, N], fr, name=f"aT{k}", tag=f"aT{k}") for k in range(KC)]

            def ln_tiles(src, dstT, keep_raw_t0=False):
                # natural-layout LN per 128-token tile, then PE-transpose to dstT
                for tt in range(N // 128):
                    x = ad.tile([128, C_S], f32, name="ln_x", tag="ln_x")
                    dma(out=x, in_=src[tt * 128:(tt + 1) * 128, :])
                    st = ad.tile([128, nc.vector.BN_STATS_DIM], f32, name="ln_st", tag="ln_st")
                    nc.vector.bn_stats(out=st, in_=x)
                    mv = ad.tile([128, 2], f32, name="ln_mv", tag="ln_mv")
                    nc.vector.bn_aggr(out=mv, in_=st)
                    sd = ad.tile([128, 1], f32, name="ln_sd", tag="ln_sd")
                    nc.scalar.activation(out=sd, in_=mv[:, 1:2], func=AF.Sqrt,
                                         bias=eps_t, scale=1.0)
                    rstd = ad.tile([128, 1], f32, name="ln_rstd", tag="ln_rstd")
                    nc.vector.reciprocal(out=rstd, in_=sd)
                    y = ad.tile([128, C_S], f32, name="ln_y", tag="ln_y")
                    nc.vector.tensor_scalar(out=y, in0=x, scalar1=mv[:, 0:1],
                                            scalar2=rstd, op0=ALU.subtract,
                                            op1=ALU.mult)
                    for k in range(KC):
                        pt = psm()
                        nc.tensor.transpose(pt, y[:, k * 128:(k + 1) * 128], ident)
                        copy_alt(dstT[k][:, tt * 128:(tt + 1) * 128], pt)
                    if keep_raw_t0 and tt == 0:
                        for k in range(KC):
                            pt = psm()
                            nc.tensor.transpose(pt, x[:, k * 128:(k + 1) * 128], ident)
                            copy_alt(siT[k], pt)

            ln_tiles(p["s_in"][:, :], sT, keep_raw_t0=True)
            ln_tiles(p["a_in"][:, :], lnaT)

            # sig/lin chains in transposed layout: out [c_out chunk, tok]
            for m in range(KC):
                for half in range(NHALF):
                    sl = slice(half * LW, (half + 1) * LW)
                    p1 = pbig()
                    for k in range(KC):
                        nc.tensor.matmul(p1[:, 0:LW], lhsT=ws_sb[:, k, m * 128:(m + 1) * 128],
                                         rhs=sT[k][:, sl],
                                         start=(k == 0), stop=(k == KC - 1))
                    sig = ad.tile([128, LW], f32, name="sig", tag="sig")
                    nc.scalar.activation(out=sig, in_=p1[:, 0:LW], func=AF.Sigmoid,
                                         bias=bs_sb[:, m:m + 1], scale=1.0)
                    p2t = pbig()
                    for k in range(KC):
                        nc.tensor.matmul(p2t[:, 0:LW], lhsT=wns_sb[:, k, m * 128:(m + 1) * 128],
                                         rhs=sT[k][:, sl],
                                         start=(k == 0), stop=(k == KC - 1))
                    nc.vector.tensor_mul(out=aT[m][:, sl], in0=sig, in1=lnaT[m][:, sl])
                    nc.vector.tensor_add(out=aT[m][:, sl], in0=aT[m][:, sl].bitcast(f32),
                                         in1=p2t[:, 0:LW])

            # kT[mc] = (a @ wk)^T ; qT/gT for own block (first 128 rotated tokens)
            for m in range(MC4):
                for half in range(NHALF):
                    sl = slice(half * LW, (half + 1) * LW)
                    pk = pbig()
                    for k in range(KC):
                        nc.tensor.matmul(pk[:, 0:LW], lhsT=wk_sb[:, k, m * 128:(m + 1) * 128],
                                         rhs=aT[k][:, sl],
                                         start=(k == 0), stop=(k == KC - 1))
                    copy_alt(kT[m][:, sl], pk[:, 0:LW])
                pq = psm()
                for k in range(KC):
                    nc.tensor.matmul(pq, lhsT=wq_sb[:, k, m * 128:(m + 1) * 128],
                                     rhs=aT[k][:, 0:IB],
                                     start=(k == 0), stop=(k == KC - 1))
                nc.scalar.activation(out=qT[m], in_=pq, func=AF.Identity,
                                     bias=bq_sb[:, m:m + 1], scale=1.0)
                pg = psm()
                for k in range(KC):
                    nc.tensor.matmul(pg, lhsT=wg_sb[:, k, m * 128:(m + 1) * 128],
                                     rhs=aT[k][:, 0:IB],
                                     start=(k == 0), stop=(k == KC - 1))
                nc.scalar.activation(out=gT[m], in_=pg, func=AF.Sigmoid, scale=1.0)

            # V natural [j, hdp]: lhsT = aT column chunk (stationary), rhs = wv
            for j in range(NJC):
                pv = pbig()
                for k in range(KC):
                    nc.tensor.matmul(pv[:, 0:HDP], lhsT=aT[k][:, j * 128:(j + 1) * 128],
                                     rhs=wv_sb[:, k, :],
                                     start=(k == 0), stop=(k == KC - 1))
                copy_alt(V[j], pv[:, 0:HDP])

        # ============ Phase C: z stream ============
        # Stream order (host-permuted): chunk t holds 512-col pieces of all
        # four i-quarters; quarter s lands at psum col-group 32s so the
        # [128, .] staging/DRAM round-trip uses full-partition DMAs.
        sync_point()
        SGT = 8                                   # stream-chunks per staging tile
        NT = NZC                                  # 64 stream chunks
        with tc.tile_pool(name="zp", bufs=3) as zp, \
             tc.tile_pool(name="sqp", bufs=2) as sqp, \
             tc.tile_pool(name="stg", bufs=2) as stg:
            for g in range(NT // SGT):            # 8 staging groups
                stage = stg.tile([128, SGT * NMM], bf16, name="stage", tag="stage")
                for q in range(SGT):
                    t = g * SGT + q
                    zt = zp.tile([C_Z, ZCH], zdt, name="zt", tag="zt")
                    dma(out=zt,
                        in_=p["z_t"][:, t * ZCH:(t + 1) * ZCH])
                    zs = sqp.tile([C_Z, ZCH], zdt, name="zs", tag="zs")
                    zsq_i = nc.scalar.activation(out=zs, in_=zt, func=AF.Square)
                    if g == 0:
                        pin(zsq_i, ET.Activation)
                    pstat = ps_stat.tile([128, NMM], f32, name="pstat", tag="pstat")
                    for s in range(4):            # i-quarter -> psum col group
                        sl = slice(s * NMM, (s + 1) * NMM)
                        mm1 = nc.tensor.matmul(pstat[32 * s:32 * s + 32, :], lhsT=wba_sb,
                                               rhs=zt[:, sl], start=True, stop=False,
                                               tile_position=(0, 32 * s))
                        mm2 = nc.tensor.matmul(pstat[32 * s:32 * s + 32, :], lhsT=sqa_sb,
                                               rhs=zs[:, sl], start=False, stop=True,
                                               tile_position=(0, 32 * s))
                        if g == 0:
                            pin(mm1, ET.PE)
                            pin(mm2, ET.PE)
                    copy_alt(stage[:, q * NMM:(q + 1) * NMM], pstat, pin_it=(g == 0))
                dma(out=braw[:, g * SGT * NMM:(g + 1) * SGT * NMM],
                    in_=stage)

        # ============ Phase D: rs tile ============
        sync_point()
        p2 = ctx.enter_context(tc.tile_pool(name="p2", bufs=1))
        att = ctx.enter_context(tc.tile_pool(name="att", bufs=3))

        IJ4 = IJ // 4

        def row_view(r):
            # bias plane for stats-row r: partition p=i reads DRAM row
            # 32*(i//32)+r, cols (i%32)*1024 + j  (see phase C layout)
            base = braw[:, :]
            return bass.AP(
                tensor=base.tensor,
                offset=base.offset + r * IJ4,
                ap=[[32 * IJ4, 4], [N, 32], [1, N]],
            )

        S = p2.tile([IB, N], bf16, name="Srow", tag="Srow")
        dma(out=S, in_=row_view(16))
        Q = p2.tile([IB, N], bf16, name="Qrow", tag="Qrow")
        dma(out=Q, in_=row_view(17))
        m_t = p2.tile([IB, N], f32, name="mrow", tag="mrow")
        pin(nc.vector.tensor_scalar_mul(out=m_t, in0=S, scalar1=1.0 / C_Z), ET.DVE)
        msq = p2.tile([IB, N], f32, name="msq", tag="msq")
        nc.vector.tensor_mul(out=msq, in0=m_t, in1=m_t)
        var = p2.tile([IB, N], f32, name="var", tag="var")
        nc.vector.tensor_scalar_mul(out=var, in0=Q, scalar1=1.0 / C_Z)
        nc.vector.tensor_tensor(out=var, in0=var, in1=msq, op=ALU.subtract)
        sd2 = p2.tile([IB, N], f32, name="sd2", tag="sd2")
        pin(nc.scalar.activation(out=sd2, in_=var, func=AF.Sqrt, bias=eps_t,
                                 scale=1.0), ET.Activation)
        rs = p2.tile([IB, N], f32, name="rs", tag="rs")
        nc.vector.reciprocal(out=rs, in_=sd2)

        # ============ Phase E: attention per head ============
        oT = [p2.tile([128, IB], f32, name=f"oT{m}", tag=f"oT{m}") for m in range(MC4)]
        ops = None
        for h in range(H):
            c4, r = h // 4, 32 * (h % 4)
            bh = att.tile([IB, N], bf16, name="bh", tag="bh")
            dma(out=bh, in_=row_view(h))
            X = att.tile([IB, N], f32, name="X", tag="X")
            x_i = nc.vector.tensor_mul(out=X, in0=bh, in1=rs)
            if h == 0:
                pin(x_i, ET.DVE)
            E = att.tile([IB, N], f32, name="E", tag="E")
            dens = att.tile([IB, max(NHALF, 2)], f32, name="dens", tag="dens")
            for half in range(NHALF):
                sl = slice(half * LW, (half + 1) * LW)
                Lp = pbig()
                lm = nc.tensor.matmul(Lp[0:IB, 0:LW], lhsT=qT[c4][r:r + DP, :],
                                      rhs=kT[c4][r:r + DP, sl],
                                      start=True, stop=True, tile_position=(r, 0))
                if h == 0:
                    pin(lm, ET.PE)
                L2 = att.tile([IB, LW], f32, name="L2", tag="L2")
                nc.vector.tensor_add(out=L2, in0=Lp[0:IB, 0:LW], in1=X[:, sl])
                e_i = nc.scalar.activation(out=E[:, sl], in_=L2, func=AF.Exp,
                                           accum_out=dens[:, half:half + 1])
                if h == 0:
                    pin(e_i, ET.Activation)
            den = att.tile([IB, 1], f32, name="den", tag="den")
            if NHALF == 2:
                nc.vector.tensor_add(out=den, in0=dens[:, 0:1], in1=dens[:, 1:2])
            else:
                nc.vector.tensor_copy(out=den, in_=dens[:, 0:1])
            rden = att.tile([IB, 1], f32, name="rden", tag="rden")
            nc.vector.reciprocal(out=rden, in_=den)
            nc.vector.tensor_scalar_mul(out=E, in0=E, scalar1=rden)
            ATs = att.tile([IB, N], f32, name="ATs", tag="ATs")
            if h % 4 == 0:
                ops = ps_o.tile([128, IB], f32, name="pso", tag="pso")
            for jc in range(NJC):
                sl = slice(jc * 128, (jc + 1) * 128)
                Tp = psm()
                tr_i = nc.tensor.transpose(Tp, E[:, sl], ident)
                copy_alt(ATs[:, sl], Tp, pin_it=(h == 0))
                av_i = nc.tensor.matmul(ops[r:r + DP, :], lhsT=V[jc][:, DP * h:DP * h + DP],
                                        rhs=ATs[:, sl], start=(jc == 0), stop=(jc == NJC - 1),
                                        tile_position=(0, r))
                if h == 0:
                    pin(tr_i, ET.PE)
                    pin(av_i, ET.PE)
            if h % 4 == 3:
                nc.vector.tensor_copy(out=oT[c4], in_=ops)

        # ============ Phase F: gates + output projection ============
        og = [p2.tile([128, IB], fr, name=f"og{m}", tag=f"og{m}") for m in range(MC4)]
        for m in range(MC4):
            nc.vector.tensor_mul(out=og[m], in0=oT[m], in1=gT[m])
        pout = ps_big.tile([128, NMM], f32, name="psb", tag="psb")
        for m in range(MC4):
            nc.tensor.matmul(pout[0:IB, 0:C_S], lhsT=og[m], rhs=wo_sb[:, m, :],
                             start=(m == 0), stop=(m == MC4 - 1))
        psg = ps_big.tile([128, NMM], f32, name="psb", tag="psb")
        for k in range(KC):
            nc.tensor.matmul(psg[0:IB, 0:C_S], lhsT=siT[k], rhs=wsg_sb[:, k, :],
                             start=(k == 0), stop=(k == KC - 1))
        sgl = p2.tile([IB, C_S], f32, name="sgl", tag="sgl")
        nc.vector.tensor_add(out=sgl, in0=psg[0:IB, 0:C_S], in1=bsg_bc)
        sg = p2.tile([IB, C_S], f32, name="sg", tag="sg")
        nc.scalar.activation(out=sg, in_=sgl, func=AF.Sigmoid, scale=1.0)
        fin = p2.tile([IB, C_S], f32, name="fin", tag="fin")
        nc.vector.tensor_mul(out=fin, in0=pout[0:IB, 0:C_S], in1=sg)
        dma(out=p["out"][:, :], in_=fin)


def _prep_host(inputs):
    """Fold weights, pad heads, shard + rotate per core."""
    i = {k: np.asarray(v, dtype=np.float32) for k, v in inputs.items()}
    lnsw = i["adaln_lns_w"]                      # [C_S]
    w_ws = np.ascontiguousarray(lnsw[:, None] * i["adaln_ws"])
    w_wns = np.ascontiguousarray(lnsw[:, None] * i["adaln_wns"])

    def pad_heads(w, scale=1.0):                 # [C_S, H*D] -> [C_S, H*DP]
        wp = np.zeros((C_S, HDP), np.float32)
        for h in range(H):
            wp[:, h * DP:h * DP + D] = w[:, h * D:(h + 1) * D] * scale
        return wp

    w_q = pad_heads(i["wq"], SCALE)
    b_q = np.zeros((HDP,), np.float32)
    for h in range(H):
        b_q[h * DP:h * DP + D] = i["bq"][h * D:(h + 1) * D] * SCALE
    w_k = pad_heads(i["wk"])
    w_v = pad_heads(i["wv"])
    w_g = pad_heads(i["wg"])
    w_o = np.zeros((HDP, C_S), np.float32)
    for h in range(H):
        w_o[h * DP:h * DP + D, :] = i["wo"][h * D:(h + 1) * D, :]

    wbp = i["lnb_w"][:, None] * i["wb"]          # [C_Z, H]
    wbc = wbp - wbp.mean(axis=0, keepdims=True)  # fold LN mean-centering
    wb_aug = np.zeros((C_Z, 32), np.float32)
    wb_aug[:, :H] = wbc
    wb_aug[:, 16] = 1.0                          # column sum of z
    sq_aug = np.zeros((C_Z, 32), np.float32)
    sq_aug[:, 17] = 1.0                          # column sum of z^2
    if USE_ZBF16:
        import ml_dtypes
        wb_aug = wb_aug.astype(ml_dtypes.bfloat16)
        sq_aug = sq_aug.astype(ml_dtypes.bfloat16)

    z0 = i["z_ij"][0]                            # [N, N, C_Z]
    zT_full = np.ascontiguousarray(z0.transpose(2, 0, 1))  # [C_Z, N(i), N(j)]

    in_maps = []
    for c in range(NCORES):
        i0 = c * IB
        ridx = (np.arange(N) + i0) % N           # token rotation
        zc = zT_full[:, i0:i0 + IB, :][:, :, ridx]          # [C_Z, IB, N]
        zarr = zc.reshape(C_Z, 4, IJ // (4 * NMM), NMM).transpose(0, 2, 1, 3)
        zarr = np.ascontiguousarray(zarr.reshape(C_Z, IJ))
        if USE_ZBF16:
            import ml_dtypes
            zarr = zarr.astype(ml_dtypes.bfloat16)
        in_maps.append({
            "z_t": zarr,
            "a_in": np.ascontiguousarray(i["a_i"][0][ridx]),
            "s_in": np.ascontiguousarray(i["s_i"][0][ridx]),
            "w_ws": w_ws, "w_wns": w_wns, "b_s": i["adaln_bs"],
            "w_q": w_q, "b_q": b_q, "w_k": w_k, "w_v": w_v, "w_g": w_g,
            "wb_aug": wb_aug, "sq_aug": sq_aug,
            "w_o": w_o, "w_sg": i["ws"], "b_sg": i["bs"],
        })
    return in_maps


LAST_EXEC_NS = None


def _run_timed(nc, in_maps, n_iters=6):
    """Execute via PJRT with device-resident inputs; time repeated calls.

    Returns (results, best_exec_seconds). Mirrors bass2jax.run_bass_via_pjrt's
    multi-core branch but without donation so the executable can be re-run on
    the same buffers.
    """
    import time as _time

    import jax
    from jax.sharding import Mesh, PartitionSpec
    from jax.experimental.shard_map import shard_map
    from concourse import mybir as _mb
    from concourse.bass2jax import (_bass_exec_p, install_neuronx_cc_hook,
                                    partition_id_tensor)

    install_neuronx_cc_hook()
    n_cores = len(in_maps)
    pname = nc.partition_id_tensor.name if nc.partition_id_tensor else None

    in_names, out_names, out_avals, zero_outs = [], [], [], []
    for alloc in nc.m.functions[0].allocations:
        if not isinstance(alloc, _mb.MemoryLocationSet):
            continue
        name = alloc.memorylocations[0].name
        if alloc.kind == "ExternalInput":
            if name != pname:
                in_names.append(name)
        elif alloc.kind == "ExternalOutput":
            out_names.append(name)
            shape = tuple(alloc.tensor_shape)
            dtype = _mb.dt.np(alloc.dtype)
            out_avals.append(jax.core.ShapedArray(shape, dtype))
            zero_outs.append(np.zeros(shape, dtype))
    n_params = len(in_names)
    all_in_names = in_names + out_names
    if pname is not None:
        all_in_names = all_in_names + [pname]

    def _body(*args):
        operands = list(args)
        if pname is not None:
            operands.append(partition_id_tensor())
        outs = _bass_exec_p.bind(
            *operands,
            out_avals=tuple(out_avals),
            in_names=tuple(all_in_names),
            out_names=tuple(out_names),
            lowering_input_output_aliases=(),
            sim_require_finite=True,
            sim_require_nnan=True,
            nc=nc,
        )
        return tuple(outs)

    devices = jax.devices()[:n_cores]
    mesh = Mesh(np.asarray(devices), ("core",))
    in_specs = (PartitionSpec("core"),) * (n_params + len(out_names))
    out_specs = (PartitionSpec("core"),) * len(out_names)
    fn = jax.jit(shard_map(_body, mesh=mesh, in_specs=in_specs,
                           out_specs=out_specs, check_rep=False),
                 keep_unused=True)

    concat_in = [
        np.concatenate([np.asarray(in_maps[c][nm]) for c in range(n_cores)], axis=0)
        for nm in in_names
    ]
    concat_zeros = [
        np.zeros((n_cores * z.shape[0], *z.shape[1:]), z.dtype) for z in zero_outs
    ]
    sharding = jax.sharding.NamedSharding(mesh, PartitionSpec("core"))
    dev_in = [jax.device_put(a, sharding) for a in concat_in]
    dev_zero = [jax.device_put(a, sharding) for a in concat_zeros]

    out_arrs = fn(*dev_in, *dev_zero)      # warmup + compile
    jax.block_until_ready(out_arrs)
    best = float("inf")
    for _ in range(n_iters):
        t0 = _time.perf_counter()
        r = fn(*dev_in, *dev_zero)
        jax.block_until_ready(r)
        best = min(best, _time.perf_counter() - t0)
    out_arrs = r
    results = [
        {nm: np.asarray(out_arrs[i]).reshape(n_cores, *out_avals[i].shape)[c]
         for i, nm in enumerate(out_names)}
        for c in range(n_cores)
    ]
    return results, best


def kernel(**inputs) -> np.ndarray:
    global LAST_EXEC_NS
    if "nc" not in _CACHED:
        _CACHED["nc"] = _build_program()
    nc = _CACHED["nc"]
    in_maps = _prep_host(inputs)
    if os.environ.get("KERNEL_TIMED", "0") == "1":
        outs, best_s = _run_timed(nc, in_maps)
        LAST_EXEC_NS = int(best_s * 1e9)
    else:
        res = run_bass_kernel_spmd(nc, in_maps, list(range(NCORES)))
        LAST_EXEC_NS = getattr(res, "exec_time_ns", None)
        outs = res.results
    full = np.concatenate([outs[c]["out"] for c in range(NCORES)], axis=0)
    return full[None, :, :].astype(np.float32)

